# revision 1
# baseline (speedup 1.0000x reference)
"""Trainium2 Bass kernel for nn_Attention_21895743275585.

Reference computation (per batch b of 4):
  qkv = w_qkv @ x_flat            # 1x1 conv, x_flat [C=256, N=2304]
  q,k l2-normalized per (head, n) along dim_head=64; SCALE=10
  sim = 10 * qhat^T khat per head; attn = softmax(sim, axis=-1)
  out = attn @ v; final = w_out @ out_inner + b_out

Sharding: 8 cores = (batch b, head-half). Each core handles 4 of the 8 heads
of one batch; host sums the partial output projections (2 halves x 2 head
pairs per batch; bias is fed only to half 0 / pair 0).

On-core layout ([partition, free]):
  q,k "channels-major" [d, n] pairs: tile m in {q01,q23,k01,k23} = [128, N]
  v transposed [n, d] per j-tile (from a separate x^T @ w_v^T matmul) with a
  ones column appended so the E@v matmul also yields softmax denominators.
  sim^T chunk [j, i] = k^T q in PSUM (two heads row-packed via tile_position);
  ACT does exp(PSUM)->SBUF in [128, 1024] instructions (no max subtraction
  needed: |sim|<=10 exactly since q,k are unit vectors).
  1/sqrt and 1/x are computed as exp(-0.5 ln x) / exp(-ln x) -- Ln and Exp
  share one ACT table set (pinned to natural_log_exp_and_others).
  Norm rows live at partition bases {0,32,64,96} of [128, N] tiles (engine
  SBUF APs must start at partition 0/32/64/96); [1,N]->[64,N] partition
  broadcasts bounce through small internal DRAM tensors (DRAM APs allow a
  step-0 partition dim).
"""

import math

import numpy as np

B, C, H, W = 4, 256, 48, 48
HEADS, DIM_HEAD, SCALE = 8, 64, 10.0
INNER = HEADS * DIM_HEAD
N = H * W                      # 2304
NJ = N // 128                  # 18 j-tiles
CHUNKS = [(0, 512), (512, 512), (1024, 512), (1536, 512), (2048, 256)]
EPS = 1e-12

WD_NAME = "bf16"               # working dtype: "bf16" | "f32r" | "f32"

# DVE-exp offload: these j-tiles' softmax exps run on the Vector engine via
# a 2-op Schraudolph (bits + mantissa-correction) custom op, relieving the
# bottleneck ACT engine (~1.15us/tile) for ~2.4us of idle DVE time.
DVE_JTS = {0: (5, 12), 1: (5, 12)}
LOG2E = 1.4426950408889634
A_EXP = float(2 ** 23) * LOG2E
B_EXP = float(127 * 2 ** 23) + 0.5
MASK_C = float(np.int32(0x007FFFFF).view(np.float32))
GAMMA = 0.235

_CACHE = {}


def _register_exp_ops():
    """Register the 2-op Schraudolph exp into concourse's custom-DVE tables
    (runtime registration; shas computed on the fly)."""
    import concourse.dve_ops as dops
    if "EXP_BITS_ANT" in dops.CUSTOM_DVE_SPECS:
        return {"bits": dops._EXP_BITS_ANT, "fix": dops._EXP_FIX_ANT}
    from concourse.dve_spec import Spec, Src0, C0, C1, C2, AluOp, Bin, lower
    from concourse.dve_uop import DveOpSpec
    from concourse.dve_ops import DveOp

    def mk(name, spec):
        shas = {}
        for ver in ("v3", "v4"):
            try:
                sp = DveOpSpec(name=name, opcode=1,
                               uops=lower(spec, ver=ver), rd1_en=False)
                shas[ver] = sp.sha(ver)
            except Exception:
                pass
        op = DveOp(name, spec, subdim=False, uops_sha=shas)
        row = max(dops._SUB_OPCODE_FOR_NAME.values()) + 1
        assert row < 0x20
        dops.OPS.append(op)
        dops._SUB_OPCODE_FOR_NAME[op.name] = row
        dops.CUSTOM_DVE_SPECS[op.name] = op.spec
        return op

    def ref_bits(in0, in1, c0, c1, c2):
        t = in0.astype(np.float32) * np.float32(c0) + np.float32(c1)
        return t.astype(np.int32)

    spec_bits = Spec(body=Src0 * C0 + C1, reference=ref_bits)

    _and = Bin(AluOp.BITWISE_AND, Src0, C0)
    _u = Bin(AluOp.BITWISE_OR, _and, C1)
    _f = _u - C1
    _c = _f * (C1 - _f) * C2 + C1

    def ref_fix(in0, in1, c0, c1, c2):
        bits = np.asarray(in0, np.float32).view(np.int32)
        m = bits & 0x007FFFFF
        u = (m | 0x3F800000).astype(np.int32).view(np.float32)
        f = u - np.float32(c1)
        c = f * (np.float32(c1) - f) * np.float32(c2) + np.float32(c1)
        return np.asarray(in0, np.float32).view(np.float32) * c

    spec_fix = Spec(body=Src0 * _c, reference=ref_fix)

    dops._EXP_BITS_ANT = mk("EXP_BITS_ANT", spec_bits)
    dops._EXP_FIX_ANT = mk("EXP_FIX_ANT", spec_fix)
    return {"bits": dops._EXP_BITS_ANT, "fix": dops._EXP_FIX_ANT}


def _pin_act_tables():
    """Force every activation onto the natural_log_exp_and_others set so the
    whole kernel needs exactly one ACT table load (Ln+Exp share that set)."""
    import concourse.bacc as bacc_mod
    if getattr(bacc_mod, "_act_tables_pinned", False):
        return
    orig = bacc_mod.get_activation_tables

    def patched(arch):
        t = orig(arch)
        keep = "natural_log_exp_and_others"
        if keep in t:
            return {name: (funcs if name == keep else set())
                    for name, funcs in t.items()}
        return t

    bacc_mod.get_activation_tables = patched
    bacc_mod._act_tables_pinned = True


def _build(wd_name):
    import concourse.bass as bass
    import concourse.tile as tile
    from concourse import bacc, mybir

    _pin_act_tables()
    expops = _register_exp_ops()

    F32 = mybir.dt.float32
    I32 = mybir.dt.int32
    F32R = mybir.dt.float32r
    WD = mybir.dt.bfloat16 if wd_name == "bf16" else F32

    def mc(ap):
        # matmul operand cast for the fast-fp32 PE path
        return ap.bitcast(F32R) if wd_name == "f32r" else ap

    Ln = mybir.ActivationFunctionType.Ln
    Exp = mybir.ActivationFunctionType.Exp
    ActCopy = mybir.ActivationFunctionType.Copy

    nc = bacc.Bacc("TRN2", target_bir_lowering=False, debug=False,
                   enable_asserts=False, num_devices=8)
    x2 = nc.dram_tensor("x2", [2, 128, N], WD, kind="ExternalInput").ap()
    wqk = nc.dram_tensor("wqk", [2, 128, 512], WD, kind="ExternalInput").ap()
    wvT = nc.dram_tensor("wvT", [2, 128, 256], WD, kind="ExternalInput").ap()
    woT = nc.dram_tensor("woT", [2, 128, 256], WD, kind="ExternalInput").ap()
    bias = nc.dram_tensor("bias", [2, 128, 1], F32, kind="ExternalInput").ap()
    ones8 = nc.dram_tensor("ones8", [128, 9], WD, kind="ExternalInput").ap()
    # output: per head-pair partial projections, summed on host
    y = nc.dram_tensor("y", [2, 2, 128, N], F32, kind="ExternalOutput").ap()
    # internal DRAM bounce rows for partition broadcasts
    rsd = nc.dram_tensor("rsd", [8, N], F32, kind="Internal").ap()
    rsdd = nc.dram_tensor("rsdd", [4, N], F32, kind="Internal").ap()

    def bcast_row(dram_row_ap, dst_ap, parts):
        src = bass.AP(tensor=dram_row_ap.tensor, offset=dram_row_ap.offset,
                      ap=[[0, parts]] + list(dram_row_ap.ap))
        nc.sync.dma_start(dst_ap, src)

    # m tile -> norm-row base index a: q01->0, k01->1, q23->2, k23->3
    M_OF = [(0, 0), (2, 1), (1, 2), (3, 3)]

    with tile.TileContext(nc) as tc:
        with tc.tile_pool(name="persist", bufs=1) as P, \
             tc.tile_pool(name="bcast", bufs=2) as RSB, \
             tc.tile_pool(name="sq", bufs=3) as SQ, \
             tc.tile_pool(name="esb", bufs=12) as ESB, \
             tc.tile_pool(name="ib", bufs=3) as IB, \
             tc.tile_pool(name="yst", bufs=3) as YST, \
             tc.tile_pool(name="psf", bufs=2, space="PSUM") as PSF:

            # ---- load inputs ----
            x_sb = [P.tile([128, N], WD, tag=f"x{c}", name=f"x{c}")
                    for c in range(2)]
            wqk_sb = [P.tile([128, 512], WD, tag=f"wqk{c}", name=f"wqk{c}")
                      for c in range(2)]
            wvT_sb = [P.tile([128, 256], WD, tag=f"wvT{c}", name=f"wvT{c}")
                      for c in range(2)]
            woT_sb = [P.tile([128, 256], WD, tag=f"woT{c}", name=f"woT{c}")
                      for c in range(2)]
            bias_sb = [P.tile([128, 1], F32, tag=f"bias{c}", name=f"bias{c}")
                       for c in range(2)]
            ones8_sb = P.tile([128, 9], WD, tag="ones8", name="ones8")
            for c in range(2):
                nc.sync.dma_start(x_sb[c][:, 0:N // 2], x2[c][:, 0:N // 2])
                nc.sync.dma_start(wqk_sb[c][:, :], wqk[c])
            for c in range(2):
                nc.sync.dma_start(x_sb[c][:, N // 2:N],
                                  x2[c][:, N // 2:N])
                nc.sync.dma_start(wvT_sb[c][:, :], wvT[c])
                nc.sync.dma_start(woT_sb[c][:, :], woT[c])
                nc.sync.dma_start(bias_sb[c][:, :], bias[c])
            nc.sync.dma_start(ones8_sb[:, :], ones8)

            # per-partition Exp bias: ln(SCALE) on q rows (bases 0, 64),
            # 0 on k rows (bases 32, 96)
            biasln = P.tile([128, 1], F32, tag="biasln", name="biasln")
            nc.vector.memset(biasln[0:32, :], math.log(SCALE))
            nc.vector.memset(biasln[32:64, :], 0.0)
            nc.vector.memset(biasln[64:96, :], math.log(SCALE))
            nc.vector.memset(biasln[96:128, :], 0.0)

            qk_sb = [P.tile([128, N], WD, tag=f"qk{m}", name=f"qk{m}")
                     for m in range(4)]
            ss8 = P.tile([128, N], F32, tag="ss8", name="ss8")
            ln8 = P.tile([128, N], F32, tag="ln8", name="ln8")
            rs8 = P.tile([128, N], F32, tag="rs8", name="rs8")
            nc.vector.memset(ss8[:, :], 1.0)
            qhat = [P.tile([128, N], WD, tag=f"qh{p}", name=f"qh{p}")
                    for p in range(2)]
            khat = [P.tile([128, N], WD, tag=f"kh{p}", name=f"kh{p}")
                    for p in range(2)]
            vT_sb = P.tile([128, NJ, 4, 64], WD, tag="vT", name="vT")

            numer = [P.tile([128, N], WD, tag=f"nu{p}", name=f"nu{p}")
                     for p in range(2)]
            nsc = [P.tile([128, N], WD, tag=f"nsc{p}", name=f"nsc{p}")
                   for p in range(2)]
            s8 = P.tile([128, N], F32, tag="ss8", name="s8")
            s8b = P.tile([128, N], F32, tag="s8b", name="s8b")
            rsden8 = P.tile([128, N], F32, tag="rs8", name="rsden8")
            nc.vector.memset(s8[:, :], 1.0)
            nc.vector.memset(s8b[:, :], 1.0)

            # ---- phase 1: QKV projection, norms, v^T ----
            with tc.tile_pool(name="psq", bufs=2, space="PSUM") as PSQ, \
                 tc.tile_pool(name="pss", bufs=2, space="PSUM") as PSS, \
                 tc.tile_pool(name="psv", bufs=2, space="PSUM") as PSV:

                def qkv_chunk(m, a, off, cw, copy_eng, PQ, PS2,
                              pqtag, psstag):
                    base = 32 * a
                    pq = PQ.tile([128, 512], F32, tag=pqtag, name=pqtag)
                    for c in range(2):
                        nc.tensor.matmul(
                            pq[:, 0:cw],
                            mc(wqk_sb[c][:, m * 128:(m + 1) * 128]),
                            mc(x_sb[c][:, off:off + cw]),
                            start=(c == 0), stop=(c == 1))
                    if copy_eng == "act":
                        nc.scalar.activation(qk_sb[m][:, off:off + cw],
                                             pq[:, 0:cw], ActCopy)
                    else:
                        nc.vector.tensor_copy(qk_sb[m][:, off:off + cw],
                                              pq[:, 0:cw])
                    q2 = SQ.tile([128, 512], WD, tag="q2", name="q2")
                    nc.vector.tensor_mul(q2[:, 0:cw],
                                         qk_sb[m][:, off:off + cw],
                                         qk_sb[m][:, off:off + cw])
                    pss = PS2.tile([8, 512], F32, tag=psstag, name=psstag)
                    nc.tensor.matmul(pss[:, 0:cw], mc(ones8_sb[:, 0:8]),
                                     mc(q2[:, 0:cw]), start=True, stop=True)
                    if copy_eng == "act":
                        nc.scalar.activation(ss8[base:base + 2,
                                                 off:off + cw],
                                             pss[0:2, 0:cw], ActCopy)
                    else:
                        nc.vector.tensor_copy(
                            ss8[base:base + 2, off:off + cw],
                            pss[0:2, 0:cw])

                def rs_batched(p):
                    # whole-row: exactly 2 ACT instrs ahead of attention
                    b0 = 64 * p
                    sl = slice(b0, b0 + 64)
                    nc.vector.tensor_scalar_max(ss8[sl, :], ss8[sl, :],
                                                EPS * EPS)
                    nc.scalar.activation(ln8[sl, :], ss8[sl, :], Ln)
                    nc.scalar.activation(rs8[sl, :], ln8[sl, :], Exp,
                                         scale=-0.5, bias=biasln[sl, :])
                    for a in (2 * p, 2 * p + 1):
                        nc.sync.dma_start(rsd[2 * a:2 * a + 2, :],
                                          rs8[32 * a:32 * a + 2, :])

                def norm_chunk(p, off, cw, rsbq, rsbk):
                    for (dst, a, src_m, rsb) in (
                            (qhat[p], 2 * p, p, rsbq),
                            (khat[p], 2 * p + 1, 2 + p, rsbk)):
                        bcast_row(rsd[2 * a][off:off + cw],
                                  rsb[0:64, off:off + cw], 64)
                        bcast_row(rsd[2 * a + 1][off:off + cw],
                                  rsb[64:128, off:off + cw], 64)
                        nc.vector.tensor_mul(dst[:, off:off + cw],
                                             qk_sb[src_m][:, off:off + cw],
                                             rsb[:, off:off + cw])

                # pair 0, chunk-major: attention can start after chunk 0.
                # q copies on ACT, k copies on DVE to balance the two queues.
                rsbq0 = RSB.tile([128, N], F32, tag="rsb", name="rsbq0")
                rsbk0 = RSB.tile([128, N], F32, tag="rsb", name="rsbk0")
                for (off, cw) in CHUNKS:
                    qkv_chunk(0, 0, off, cw, "act", PSQ, PSS, "pq", "pss")
                    qkv_chunk(2, 1, off, cw, "act", PSQ, PSS, "pq", "pss")
                rs_batched(0)
                for (off, cw) in CHUNKS:
                    norm_chunk(0, off, cw, rsbq0, rsbk0)

                # v^T via x^T @ w_v^T (PE work; overlaps the pair-0 chain)
                for jt in range(NJ):
                    pv = PSV.tile([128, 256], F32, tag="pv", name="pv")
                    for c in range(2):
                        nc.tensor.matmul(
                            pv[:, :],
                            mc(x_sb[c][:, jt * 128:(jt + 1) * 128]),
                            mc(wvT_sb[c][:, :]),
                            start=(c == 0), stop=(c == 1))
                    nc.vector.tensor_copy(
                        vT_sb[:, jt, :, :],
                        pv.rearrange("p (h d) -> p h d", h=4))



            # ---- phase 2+3: attention, scaling, output projection ----
            with tc.tile_pool(name="pssim", bufs=2, space="PSUM") as PSSIM, \
                 tc.tile_pool(name="pso", bufs=1, space="PSUM") as PSO:

                def attention_pair(hp, chunks, jt_filler=None):
                    for (off, cw) in chunks:
                        po = PSO.tile([128, 512], F32, tag="po", name="po")
                        po_o = PSO.tile([33, 512], F32, tag="po_o",
                                        name="po_o")

                        def sim_pair(jt, ps):
                            js = slice(jt * 128, (jt + 1) * 128)
                            nc.tensor.matmul(
                                ps[:, 0:cw],
                                mc(khat[hp][0:64, js]),
                                mc(qhat[hp][0:64, off:off + cw]),
                                start=True, stop=True, tile_position=(0, 0))
                            nc.tensor.matmul(
                                ps[:, 512:512 + cw],
                                mc(khat[hp][64:128, js]),
                                mc(qhat[hp][64:128, off:off + cw]),
                                start=True, stop=True, tile_position=(64, 0))

                        def ev_group(jt, eh0, eh1):
                            st, sp = (jt == 0), (jt == NJ - 1)
                            nc.tensor.matmul(
                                po[0:64, 0:cw],
                                mc(vT_sb[:, jt, 2 * hp, :]),
                                mc(eh0),
                                start=st, stop=sp, tile_position=(0, 0),
                                skip_group_check=True)
                            nc.tensor.matmul(
                                po[64:128, 0:cw],
                                mc(vT_sb[:, jt, 2 * hp + 1, :]),
                                mc(eh1),
                                start=st, stop=sp, tile_position=(0, 64),
                                skip_group_check=True)
                            nc.tensor.matmul(
                                po_o[0:1, 0:cw],
                                mc(ones8_sb[:, 8:9]),
                                mc(eh0),
                                start=st, stop=sp, tile_position=(0, 0),
                                skip_group_check=True)
                            nc.tensor.matmul(
                                po_o[32:33, 0:cw],
                                mc(ones8_sb[:, 8:9]),
                                mc(eh1),
                                start=st, stop=sp, tile_position=(0, 32),
                                skip_group_check=True)

                        # E@v trails 3 j's behind so the next chunk's
                        # first E@v (which waits the previous chunk's po
                        # drain) never blocks early sims on the in-order PE
                        pend = []
                        for jt in range(NJ):
                            ps = PSSIM.tile([128, 1024], F32, tag="ps",
                                            name="ps")
                            sim_pair(jt, ps)
                            e = ESB.tile([128, 1024], WD, tag="e",
                                         name="e")
                            ps3 = ps.rearrange("p (b c) -> p b c", b=2)
                            e3b = e.rearrange("p (b c) -> p b c", b=2)
                            if jt in DVE_JTS[hp]:
                                ib = IB.tile([128, 1024], I32, tag="ib",
                                             name="ib")
                                ib3 = ib.rearrange("p (b c) -> p b c", b=2)
                                nc.vector._custom_dve(
                                    expops["bits"],
                                    out=ib3[:, :, 0:cw],
                                    in0=ps3[:, :, 0:cw],
                                    s0=A_EXP, s1=B_EXP)
                                nc.vector._custom_dve(
                                    expops["fix"],
                                    out=e3b[:, :, 0:cw],
                                    in0=ib3[:, :, 0:cw].bitcast(F32),
                                    s0=MASK_C, s1=1.0, imm2=-GAMMA)
                            else:
                                nc.scalar.activation(e3b[:, :, 0:cw],
                                                     ps3[:, :, 0:cw], Exp)
                            if jt_filler is not None:
                                jt_filler(jt)
                            pend.append((jt, e))
                            if len(pend) > 3:
                                j0, ee = pend.pop(0)
                                ev_group(j0, ee[:, 0:cw],
                                         ee[:, 512:512 + cw])
                        for (j0, ee) in pend:
                            ev_group(j0, ee[:, 0:cw], ee[:, 512:512 + cw])
                        # drain numerators + denominators (s rows at base 32h)
                        nc.vector.tensor_copy(numer[hp][:, off:off + cw],
                                              po[:, 0:cw])
                        dstt = s8 if hp == 0 else s8b
                        for t in range(2):
                            nc.vector.tensor_copy(
                                dstt[32 * t:32 * t + 1, off:off + cw],
                                po_o[32 * t:32 * t + 1, 0:cw])

                def outproj_pair(pr):
                    for m2 in range(2):
                        for (off, cw) in CHUNKS:
                            pf = PSF.tile([128, 512], F32, tag="pf",
                                          name="pf")
                            nc.tensor.matmul(
                                pf[:, 0:cw],
                                mc(woT_sb[pr][:, m2 * 128:(m2 + 1) * 128]),
                                mc(nsc[pr][:, off:off + cw]),
                                start=True, stop=True)
                            yt = YST.tile([128, 512], F32, tag="yt",
                                          name="yt")
                            if pr == 0:
                                nc.vector.tensor_scalar_add(
                                    yt[:, 0:cw], pf[:, 0:cw],
                                    bias_sb[m2][:, :])
                            else:
                                nc.vector.tensor_copy(yt[:, 0:cw],
                                                      pf[:, 0:cw])
                            nc.sync.dma_start(y[pr][m2][:, off:off + cw],
                                              yt[:, 0:cw])

                # hp0 chunk 0 first, then pair-1 QKV (DVE copies,
                # PSF psum slots, chunked rs on ACT) hidden inside the hp0
                # attention window, then the rest.
                # pair-1 QKV is spread one chunk at a time between hp0's
                # attention chunks so its PE matmuls never queue en masse
                # ahead of later sims on the in-order PE stream.
                rsbq1 = RSB.tile([128, N], F32, tag="rsb", name="rsbq1")
                rsbk1 = RSB.tile([128, N], F32, tag="rsb", name="rsbk1")
                for ci, (off, cw) in enumerate(CHUNKS):
                    attention_pair(0, [CHUNKS[ci]])
                    qkv_chunk(1, 2, off, cw, "dve", PSF, PSF, "pf", "pf")
                    qkv_chunk(3, 3, off, cw, "dve", PSF, PSF, "pf", "pf")
                    if ci == len(CHUNKS) - 1:
                        rs_batched(1)
                        for (off2, cw2) in CHUNKS:
                            norm_chunk(1, off2, cw2, rsbq1, rsbk1)
                # pair-0 1/s via the DVE divider, chunk by chunk between
                # hp1's attention chunks (DVE is idle there); outproj0 runs
                # in hp1's PE slack afterwards.
                rsb0 = RSB.tile([128, N], F32, tag="rsb", name="rsb0")
                rsb1 = RSB.tile([128, N], F32, tag="rsb", name="rsb1")

                def scale1_chunk(off, cw):
                    nc.vector.reciprocal_approx_fast(
                        out=rsden8[0:64, off:off + cw],
                        in_=s8b[0:64, off:off + cw])
                    for t in (2, 3):
                        nc.sync.dma_start(rsdd[t:t + 1, off:off + cw],
                                          rsden8[32 * (t - 2):
                                                 32 * (t - 2) + 1,
                                                 off:off + cw])
                    bcast_row(rsdd[2][off:off + cw],
                              rsb1[0:64, off:off + cw], 64)
                    bcast_row(rsdd[3][off:off + cw],
                              rsb1[64:128, off:off + cw], 64)
                    nc.vector.tensor_mul(nsc[1][:, off:off + cw],
                                         numer[1][:, off:off + cw],
                                         rsb1[:, off:off + cw])

                def outproj_chunk(pr, off, cw):
                    for m2 in range(2):
                        pf = PSF.tile([128, 512], F32, tag="pf", name="pf")
                        nc.tensor.matmul(
                            pf[:, 0:cw],
                            mc(woT_sb[pr][:, m2 * 128:(m2 + 1) * 128]),
                            mc(nsc[pr][:, off:off + cw]),
                            start=True, stop=True)
                        yt = YST.tile([128, 512], F32, tag="yt", name="yt")
                        if pr == 0:
                            nc.vector.tensor_scalar_add(
                                yt[:, 0:cw], pf[:, 0:cw],
                                bias_sb[m2][:, :])
                        else:
                            nc.scalar.activation(yt[:, 0:cw], pf[:, 0:cw],
                                                 ActCopy)
                        nc.sync.dma_start(y[pr][m2][:, off:off + cw],
                                          yt[:, 0:cw])

                for ci, ch in enumerate(CHUNKS):
                    attention_pair(1, [ch])
                    off, cw = ch
                    nc.vector.reciprocal_approx_fast(
                        out=rsden8[0:64, off:off + cw],
                        in_=s8[0:64, off:off + cw])
                    for t in range(2):
                        nc.sync.dma_start(rsdd[t:t + 1, off:off + cw],
                                          rsden8[32 * t:32 * t + 1,
                                                 off:off + cw])
                    bcast_row(rsdd[0][off:off + cw],
                              rsb0[0:64, off:off + cw], 64)
                    bcast_row(rsdd[1][off:off + cw],
                              rsb0[64:128, off:off + cw], 64)
                    nc.vector.tensor_mul(nsc[0][:, off:off + cw],
                                         numer[0][:, off:off + cw],
                                         rsb0[:, off:off + cw])

                # tail: all DMA-heavy scale/outproj kept OUT of the
                # ACT-saturated attention window (SBUF/DMA contention
                # uniformly slows every engine there); pipelined per chunk
                # with drains on the now-idle ACT
                for (off, cw) in CHUNKS:
                    outproj_chunk(0, off, cw)
                    scale1_chunk(off, cw)
                for (off, cw) in CHUNKS:
                    outproj_chunk(1, off, cw)

    nc.compile()
    return nc


def _get_program(wd_name=WD_NAME):
    if wd_name not in _CACHE:
        _CACHE[wd_name] = _build(wd_name)
    return _CACHE[wd_name]


def _np_wd(wd_name):
    if wd_name == "bf16":
        import ml_dtypes
        return np.dtype(ml_dtypes.bfloat16)
    return np.dtype(np.float32)


def make_in_maps(x, w_qkv, w_out, b_out, wd_name=WD_NAME):
    x = np.asarray(x, np.float32)
    w_qkv = np.asarray(w_qkv, np.float32)
    w_out = np.asarray(w_out, np.float32)
    b_out = np.asarray(b_out, np.float32)
    wd = _np_wd(wd_name)

    ones8 = np.zeros((128, 9), np.float32)
    ones8[:, 8] = 1.0
    for cc in range(8):
        lo = 64 * (cc % 2)
        ones8[lo:lo + 64, cc] = 1.0

    in_maps = []
    for core in range(8):
        b, half = core // 2, core % 2
        hsel = slice(256 * half, 256 * (half + 1))
        q_rows = np.arange(0, 512)[hsel]
        k_rows = 512 + q_rows
        v_rows = 1024 + q_rows
        wqk_h = np.ascontiguousarray(
            w_qkv[np.r_[q_rows, k_rows], :].T).reshape(2, 128, 512)
        wvT_h = np.ascontiguousarray(w_qkv[v_rows, :].T).reshape(2, 128, 256)
        woT_h = np.ascontiguousarray(w_out[:, hsel].T).reshape(2, 128, 256)
        bias_h = (b_out if half == 0 else np.zeros_like(b_out))
        in_maps.append({
            "x2": x[b].reshape(C, N).reshape(2, 128, N).astype(wd),
            "wqk": wqk_h.astype(wd),
            "wvT": wvT_h.astype(wd),
            "woT": woT_h.astype(wd),
            "bias": bias_h.reshape(2, 128, 1).astype(np.float32),
            "ones8": ones8.astype(wd),
        })
    return in_maps


def gather_output(results):
    outs = [r["y"].sum(axis=0).reshape(C, N) for r in results]
    return np.stack([
        (outs[2 * b] + outs[2 * b + 1]).reshape(C, H, W) for b in range(B)
    ]).astype(np.float32)


def run(in_maps, wd_name=WD_NAME, **kwargs):
    from concourse import bass_utils
    nc = _get_program(wd_name)
    return bass_utils.run_bass_kernel_spmd(nc, in_maps,
                                           core_ids=list(range(8)), **kwargs)


def kernel(x, w_qkv, w_out, b_out):
    in_maps = make_in_maps(x, w_qkv, w_out, b_out)
    res = run(in_maps)
    return gather_output(res.results)



# revision 6
# speedup vs baseline: 1.0809x; 1.0809x over previous
"""Trainium2 Bass kernel for nn_Attention_21895743275585.

Reference computation (per batch b of 4):
  qkv = w_qkv @ x_flat            # 1x1 conv, x_flat [C=256, N=2304]
  q,k l2-normalized per (head, n) along dim_head=64; SCALE=10
  sim = 10 * qhat^T khat per head; attn = softmax(sim, axis=-1)
  out = attn @ v; final = w_out @ out_inner + b_out

Sharding: 8 cores = (batch b, head-half). Each core handles 4 of the 8 heads
of one batch; on-core the two head-pair output projections accumulate in
PSUM, so each core emits one [256, N] partial and the host sums the two
half-cores per batch (bias fed only to half 0).

On-core layout ([partition, free]):
  q,k "channels-major" [d, n]: qk4 [128, 4(t), N] with t in {q01,k01,q23,k23}
  v transposed [n, d] per j-tile with a ones column appended (65-wide
  stationary), so each E@v matmul also emits that head's softmax
  denominator row into PSUM partition 64 -- no separate ones matmuls.
  sim^T chunk [j, i] = k^T q in PSUM (two heads row-packed via
  tile_position); softmax exp PSUM->SBUF splits between ACT (table Exp)
  and DVE (one-op Schraudolph: int16(x*184.665+16251) bitcast to bf16;
  numerator/denominator share the approximation so its ~3% jitter
  largely cancels in the softmax).
  1/sqrt for the l2 norms runs as exp(-0.5 ln x) on ACT (one table set,
  pinned); norm rows bounce through small DRAM tensors (bf16) to
  partition-broadcast, then one 2x-mode bf16 DVE multiply per (pair,
  chunk) forms qhat,khat packed [128, 4, N].
  GPSIMD (idle otherwise) takes the big memsets and the pair-1 squares /
  norm multiplies that sit inside the head-pair-0 attention window.
"""

import math

import numpy as np

B, C, H, W = 4, 256, 48, 48
HEADS, DIM_HEAD, SCALE = 8, 64, 10.0
INNER = HEADS * DIM_HEAD
N = H * W                      # 2304
NJ = N // 128                  # 18 j-tiles
CHUNKS = [(0, 512), (512, 512), (1024, 512), (1536, 512), (2048, 256)]
EPS = 1e-12

WD_NAME = "bf16"               # working dtype: "bf16" | "f32r" | "f32"

# softmax-exp engine split: these j-tiles run on DVE (one-op Schraudolph),
# the rest on ACT (table exp). Tuned so ACT ~ DVE ~ just under PE per chunk.
DVE_JTS = {0: (2, 5, 8, 11, 14, 16), 1: (2, 5, 8, 11, 14, 16)}
LOG2E = 1.4426950408889634
# fp32-bits Schraudolph constants (legacy two-op path, kept registered)
A_EXP = float(2 ** 23) * LOG2E
B_EXP = float(127 * 2 ** 23) + 0.5
MASK_C = float(np.int32(0x007FFFFF).view(np.float32))
GAMMA = 0.235
# bf16-bits one-op Schraudolph: int16(x * 128*log2e + (127*128 + 0.5 - 5.51))
# the -5.51 centers the (1+f)/2^f interpolation error to +-3.0%.
A_E16 = 128.0 * LOG2E
B_E16 = 127.0 * 128.0 + 0.5 - 5.5085

_CACHE = {}


def _register_exp_ops():
    """Register the Schraudolph exp ops into concourse's custom-DVE tables
    (runtime registration; shas computed on the fly)."""
    import concourse.dve_ops as dops
    if "EXP_B16_ANT" in dops.CUSTOM_DVE_SPECS:
        return {"bits": dops._EXP_BITS_ANT, "fix": dops._EXP_FIX_ANT,
                "b16": dops._EXP_B16_ANT}
    from concourse.dve_spec import Spec, Src0, C0, C1, C2, AluOp, Bin, lower
    from concourse.dve_uop import DveOpSpec
    from concourse.dve_ops import DveOp

    def mk(name, spec):
        shas = {}
        for ver in ("v3", "v4"):
            try:
                sp = DveOpSpec(name=name, opcode=1,
                               uops=lower(spec, ver=ver), rd1_en=False)
                shas[ver] = sp.sha(ver)
            except Exception:
                pass
        op = DveOp(name, spec, subdim=False, uops_sha=shas)
        row = max(dops._SUB_OPCODE_FOR_NAME.values()) + 1
        assert row < 0x20
        dops.OPS.append(op)
        dops._SUB_OPCODE_FOR_NAME[op.name] = row
        dops.CUSTOM_DVE_SPECS[op.name] = op.spec
        return op

    def ref_bits(in0, in1, c0, c1, c2):
        t = in0.astype(np.float32) * np.float32(c0) + np.float32(c1)
        return t.astype(np.int32)

    spec_bits = Spec(body=Src0 * C0 + C1, reference=ref_bits)

    _and = Bin(AluOp.BITWISE_AND, Src0, C0)
    _u = Bin(AluOp.BITWISE_OR, _and, C1)
    _f = _u - C1
    _c = _f * (C1 - _f) * C2 + C1

    def ref_fix(in0, in1, c0, c1, c2):
        bits = np.asarray(in0, np.float32).view(np.int32)
        m = bits & 0x007FFFFF
        u = (m | 0x3F800000).astype(np.int32).view(np.float32)
        f = u - np.float32(c1)
        c = f * (np.float32(c1) - f) * np.float32(c2) + np.float32(c1)
        return np.asarray(in0, np.float32).view(np.float32) * c

    spec_fix = Spec(body=Src0 * _c, reference=ref_fix)

    def ref_b16(in0, in1, c0, c1, c2):
        # hardware: fp32 ALU result, output-stage convert to int16 (trunc)
        t = in0.astype(np.float32) * np.float32(c0) + np.float32(c1)
        return t  # CoreSim casts to the out AP dtype (int16) itself

    spec_b16 = Spec(body=Src0 * C0 + C1, reference=ref_b16)

    dops._EXP_BITS_ANT = mk("EXP_BITS_ANT", spec_bits)
    dops._EXP_FIX_ANT = mk("EXP_FIX_ANT", spec_fix)
    dops._EXP_B16_ANT = mk("EXP_B16_ANT", spec_b16)
    return {"bits": dops._EXP_BITS_ANT, "fix": dops._EXP_FIX_ANT,
            "b16": dops._EXP_B16_ANT}


def _pin_act_tables():
    """Force every activation onto the natural_log_exp_and_others set so the
    whole kernel needs exactly one ACT table load (Ln+Exp share that set)."""
    import concourse.bacc as bacc_mod
    if getattr(bacc_mod, "_act_tables_pinned", False):
        return
    orig = bacc_mod.get_activation_tables

    def patched(arch):
        t = orig(arch)
        keep = "natural_log_exp_and_others"
        if keep in t:
            return {name: (funcs if name == keep else set())
                    for name, funcs in t.items()}
        return t

    bacc_mod.get_activation_tables = patched
    bacc_mod._act_tables_pinned = True


def _build(wd_name):
    import concourse.bass as bass
    import concourse.tile as tile
    from concourse import bacc, mybir

    _pin_act_tables()
    expops = _register_exp_ops()

    F32 = mybir.dt.float32
    I16 = mybir.dt.int16
    F32R = mybir.dt.float32r
    WD = mybir.dt.bfloat16 if wd_name == "bf16" else F32

    def mc(ap):
        # matmul operand cast for the fast-fp32 PE path
        return ap.bitcast(F32R) if wd_name == "f32r" else ap

    Ln = mybir.ActivationFunctionType.Ln
    Exp = mybir.ActivationFunctionType.Exp
    ActCopy = mybir.ActivationFunctionType.Copy

    nc = bacc.Bacc("TRN2", target_bir_lowering=False, debug=False,
                   enable_asserts=False, num_devices=8)
    x2 = nc.dram_tensor("x2", [2, 128, N], WD, kind="ExternalInput").ap()
    wqk = nc.dram_tensor("wqk", [2, 128, 512], WD, kind="ExternalInput").ap()
    wvT = nc.dram_tensor("wvT", [2, 128, 256], WD, kind="ExternalInput").ap()
    woT = nc.dram_tensor("woT", [2, 128, 256], WD, kind="ExternalInput").ap()
    bias = nc.dram_tensor("bias", [2, 128, 1], F32, kind="ExternalInput").ap()
    ones8 = nc.dram_tensor("ones8", [128, 9], WD, kind="ExternalInput").ap()
    # output: both head-pair projections pre-summed in PSUM; host adds the
    # two half-cores per batch. [m2, 128, N] row-blocks of out channels.
    y = nc.dram_tensor("y", [2, 128, N], WD, kind="ExternalOutput").ap()
    # internal DRAM bounce rows for partition broadcasts
    rsd = nc.dram_tensor("rsd", [8, N], WD, kind="Internal").ap()
    rsdd = nc.dram_tensor("rsdd", [4, N], F32, kind="Internal").ap()

    def bcast_row(dram_row_ap, dst_ap, parts):
        src = bass.AP(tensor=dram_row_ap.tensor, offset=dram_row_ap.offset,
                      ap=[[0, parts]] + list(dram_row_ap.ap))
        nc.sync.dma_start(dst_ap, src)

    # pair-major tile order t: 0=q01, 1=k01, 2=q23, 3=k23.
    # wqk stationary column block for t:  m = [0, 2, 1, 3][t]
    T2M = [0, 2, 1, 3]

    with tile.TileContext(nc) as tc:
        with tc.tile_pool(name="persist", bufs=1) as P, \
             tc.tile_pool(name="bcast", bufs=2) as RSB, \
             tc.tile_pool(name="sq", bufs=3) as SQ, \
             tc.tile_pool(name="esb", bufs=12) as ESB, \
             tc.tile_pool(name="yst", bufs=3) as YST, \
             tc.tile_pool(name="psf", bufs=2, space="PSUM") as PSF:

            # ---- persistent tiles ----
            x_sb = [P.tile([128, N], WD, tag=f"x{c}", name=f"x{c}")
                    for c in range(2)]
            wqk_sb = [P.tile([128, 512], WD, tag=f"wqk{c}", name=f"wqk{c}")
                      for c in range(2)]
            wvT_sb = [P.tile([128, 256], WD, tag=f"wvT{c}", name=f"wvT{c}")
                      for c in range(2)]
            woT_sb = [P.tile([128, 256], WD, tag=f"woT{c}", name=f"woT{c}")
                      for c in range(2)]
            bias_sb = [P.tile([128, 1], F32, tag=f"bias{c}", name=f"bias{c}")
                       for c in range(2)]
            ones8_sb = P.tile([128, 9], WD, tag="ones8", name="ones8")

            # chunked input DMA: chunk 0 of x + the qk weights first so the
            # first projection matmul can start ~2us in.
            for c in range(2):
                nc.sync.dma_start(x_sb[c][:, 0:512], x2[c][:, 0:512])
            for c in range(2):
                nc.sync.dma_start(wqk_sb[c][:, :], wqk[c])
            for (off, cw) in CHUNKS[1:]:
                for c in range(2):
                    nc.sync.dma_start(x_sb[c][:, off:off + cw],
                                      x2[c][:, off:off + cw])
            for c in range(2):
                nc.sync.dma_start(wvT_sb[c][:, :], wvT[c])
                nc.sync.dma_start(woT_sb[c][:, :], woT[c])
                nc.sync.dma_start(bias_sb[c][:, :], bias[c])
            nc.sync.dma_start(ones8_sb[:, :], ones8)

            # per-partition Exp bias: ln(SCALE) on q rows (bases 0, 64),
            # 0 on k rows (bases 32, 96)
            biasln = P.tile([128, 1], F32, tag="biasln", name="biasln")
            nc.vector.memset(biasln[0:32, :], math.log(SCALE))
            nc.vector.memset(biasln[32:64, :], 0.0)
            nc.vector.memset(biasln[64:96, :], math.log(SCALE))
            nc.vector.memset(biasln[96:128, :], 0.0)

            qk4 = P.tile([128, 4, N], WD, tag="qk4", name="qk4")
            ss8 = P.tile([128, N], F32, tag="ss8", name="ss8")
            rs8 = P.tile([128, N], WD, tag="rs8", name="rs8")
            nc.gpsimd.memset(ss8[:, :], 1.0)
            qh4 = P.tile([128, 4, N], WD, tag="qh4", name="qh4")
            vT_sb = P.tile([128, NJ, 4, 65], WD, tag="vT", name="vT")
            nc.gpsimd.memset(vT_sb[:, :, :, 64:65], 1.0)

            numer = [P.tile([128, N], WD, tag=f"nu{p}", name=f"nu{p}")
                     for p in range(2)]
            nsc = [P.tile([128, N], WD, tag=f"nsc{p}", name=f"nsc{p}")
                   for p in range(2)]
            # softmax denominators: single partition, free (head, i)
            s8 = P.tile([1, 2, N], F32, tag="s8", name="s8")
            s8b = P.tile([1, 2, N], F32, tag="s8b", name="s8b")
            rsden8 = P.tile([1, 2, N], F32, tag="rsden", name="rsden")

            def qhat(p):
                return qh4[:, 2 * p, :]

            def khat(p):
                return qh4[:, 2 * p + 1, :]

            # ---- phase 1: QKV projection, norms, v^T ----
            with tc.tile_pool(name="psq", bufs=2, space="PSUM") as PSQ, \
                 tc.tile_pool(name="pss", bufs=2, space="PSUM") as PSS, \
                 tc.tile_pool(name="psv", bufs=2, space="PSUM") as PSV:

                def qkv_pair(p, off, cw, copy_eng, PQ, PS2, pqtag, psstag):
                    """project q and k tiles of pair p for one chunk; square
                    and norm-sum them."""
                    for ti in range(2):
                        t = 2 * p + ti
                        m = T2M[t]
                        pq = PQ.tile([128, 512], F32, tag=pqtag, name=pqtag)
                        for c in range(2):
                            nc.tensor.matmul(
                                pq[:, 0:cw],
                                mc(wqk_sb[c][:, m * 128:(m + 1) * 128]),
                                mc(x_sb[c][:, off:off + cw]),
                                start=(c == 0), stop=(c == 1))
                        if copy_eng == "act":
                            nc.scalar.activation(qk4[:, t, off:off + cw],
                                                 pq[:, 0:cw], ActCopy)
                        else:
                            nc.vector.tensor_copy(qk4[:, t, off:off + cw],
                                                  pq[:, 0:cw])
                    q2 = SQ.tile([128, 2, 512], WD, tag="q2", name="q2")
                    sq_eng = nc.vector if p == 0 else nc.gpsimd
                    sq_eng.tensor_mul(q2[:, :, 0:cw],
                                      qk4[:, 2 * p:2 * p + 2, off:off + cw],
                                      qk4[:, 2 * p:2 * p + 2, off:off + cw])
                    for ti in range(2):
                        base = 32 * (2 * p + ti)
                        pss = PS2.tile([128, 512], F32, tag=psstag,
                                       name=psstag)
                        nc.tensor.matmul(pss[0:2, 0:cw],
                                         mc(ones8_sb[:, 0:2]),
                                         mc(q2[:, ti, 0:cw]),
                                         start=True, stop=True)
                        if copy_eng == "act":
                            nc.scalar.activation(
                                ss8[base:base + 2, off:off + cw],
                                pss[0:2, 0:cw], ActCopy)
                        else:
                            nc.vector.tensor_copy(
                                ss8[base:base + 2, off:off + cw],
                                pss[0:2, 0:cw])

                def rs_chunk(p, off, cw):
                    # rs = exp(-0.5 ln(ss) + biasln) on the packed norm rows
                    b0 = 64 * p
                    sl = slice(b0, b0 + 34)
                    lnq = SQ.tile([64, 512], F32, tag="lnq", name="lnq")
                    nc.scalar.activation(lnq[0:34, 0:cw], ss8[sl, off:off + cw],
                                         Ln)
                    nc.scalar.activation(rs8[sl, off:off + cw],
                                         lnq[0:34, 0:cw], Exp,
                                         scale=-0.5, bias=biasln[sl, :])
                    for a in (2 * p, 2 * p + 1):
                        nc.sync.dma_start(rsd[2 * a:2 * a + 2, off:off + cw],
                                          rs8[32 * a:32 * a + 2, off:off + cw])

                def norm_chunk(p, off, cw, rsbp, mul_eng):
                    # rsbp [128, 2, N] bf16: [:,0,:] q-norm rows, [:,1,:] k
                    for ti in range(2):
                        a = 2 * p + ti
                        bcast_row(rsd[2 * a][off:off + cw],
                                  rsbp[0:64, ti, off:off + cw], 64)
                        bcast_row(rsd[2 * a + 1][off:off + cw],
                                  rsbp[64:128, ti, off:off + cw], 64)
                    mul_eng.tensor_mul(qh4[:, 2 * p:2 * p + 2, off:off + cw],
                                       qk4[:, 2 * p:2 * p + 2, off:off + cw],
                                       rsbp[:, :, off:off + cw])

                # pair 0, chunk-major: attention can start after chunk 0.
                rsb0p = RSB.tile([128, 2, N], WD, tag="rsb", name="rsb0p")
                for ci, (off, cw) in enumerate(CHUNKS):
                    qkv_pair(0, off, cw, "act" if ci % 2 == 0 else "dve",
                             PSQ, PSS, "pq", "pss")
                    rs_chunk(0, off, cw)
                    norm_chunk(0, off, cw, rsb0p, nc.vector)

                # v^T via x^T @ w_v^T (PE work; overlaps the pair-0 chain)
                for jt in range(NJ):
                    pv = PSV.tile([128, 256], F32, tag="pv", name="pv")
                    for c in range(2):
                        nc.tensor.matmul(
                            pv[:, :],
                            mc(x_sb[c][:, jt * 128:(jt + 1) * 128]),
                            mc(wvT_sb[c][:, :]),
                            start=(c == 0), stop=(c == 1))
                    if jt % 2 == 0:
                        nc.vector.tensor_copy(
                            vT_sb[:, jt, :, 0:64],
                            pv.rearrange("p (h d) -> p h d", h=4))
                    else:
                        nc.scalar.activation(
                            vT_sb[:, jt, :, 0:64],
                            pv.rearrange("p (h d) -> p h d", h=4), ActCopy)

            # ---- phase 2+3: attention, scaling, output projection ----
            with tc.tile_pool(name="pssim", bufs=2, space="PSUM") as PSSIM, \
                 tc.tile_pool(name="pso", bufs=1, space="PSUM") as PSO:

                def attention_chunk(hp, off, cw, filler=None):
                    po = PSO.tile([128, 1024], F32, tag="po", name="po")

                    def sim_pair(jt, ps):
                        js = slice(jt * 128, (jt + 1) * 128)
                        nc.tensor.matmul(
                            ps[:, 0:cw],
                            mc(khat(hp)[0:64, js]),
                            mc(qhat(hp)[0:64, off:off + cw]),
                            start=True, stop=True, tile_position=(0, 0))
                        nc.tensor.matmul(
                            ps[:, 512:512 + cw],
                            mc(khat(hp)[64:128, js]),
                            mc(qhat(hp)[64:128, off:off + cw]),
                            start=True, stop=True, tile_position=(64, 0))

                    def ev_group(jt, eh0, eh1):
                        # 65-wide stationary: rows 0:64 = attn @ v, row 64 =
                        # softmax denominator (ones column in vT)
                        st, sp = (jt == 0), (jt == NJ - 1)
                        nc.tensor.matmul(
                            po[0:65, 0:cw],
                            mc(vT_sb[:, jt, 2 * hp, :]),
                            mc(eh0),
                            start=st, stop=sp, skip_group_check=True)
                        nc.tensor.matmul(
                            po[0:65, 512:512 + cw],
                            mc(vT_sb[:, jt, 2 * hp + 1, :]),
                            mc(eh1),
                            start=st, stop=sp, skip_group_check=True)

                    # E@v trails 3 j's behind so the next chunk's first E@v
                    # (which waits on the previous chunk's po drain) never
                    # blocks early sims on the in-order PE
                    pend = []
                    dve_jts = DVE_JTS[hp]
                    for jt in range(NJ):
                        ps = PSSIM.tile([128, 1024], F32, tag="ps",
                                        name="ps")
                        sim_pair(jt, ps)
                        e = ESB.tile([128, 1024], WD, tag="e", name="e")
                        ps3 = ps.rearrange("p (b c) -> p b c", b=2)
                        e3b = e.rearrange("p (b c) -> p b c", b=2)
                        if jt in dve_jts:
                            nc.vector._custom_dve(
                                expops["b16"],
                                out=e3b[:, :, 0:cw].bitcast(I16),
                                in0=ps3[:, :, 0:cw],
                                s0=A_E16, s1=B_E16)
                        else:
                            nc.scalar.activation(e3b[:, :, 0:cw],
                                                 ps3[:, :, 0:cw], Exp)
                        if filler is not None:
                            filler(jt)
                        pend.append((jt, e))
                        if len(pend) > 3:
                            j0, ee = pend.pop(0)
                            ev_group(j0, ee[:, 0:cw], ee[:, 512:512 + cw])
                    for (j0, ee) in pend:
                        ev_group(j0, ee[:, 0:cw], ee[:, 512:512 + cw])
                    # drain numerators + denominator row
                    nc.vector.tensor_copy(numer[hp][0:64, off:off + cw],
                                          po[0:64, 0:cw])
                    nc.vector.tensor_copy(numer[hp][64:128, off:off + cw],
                                          po[0:64, 512:512 + cw])
                    dstt = s8 if hp == 0 else s8b
                    po3 = po.rearrange("p (b c) -> p b c", b=2)
                    nc.vector.tensor_copy(dstt[0:1, :, off:off + cw],
                                          po3[64:65, :, 0:cw])

                # hp0 attention with pair-1 QKV+norms spread one chunk at a
                # time inside its window (PE matmuls never queue en masse
                # ahead of later sims on the in-order PE stream).
                rsb1p = RSB.tile([128, 2, N], WD, tag="rsb", name="rsb1p")
                for ci, (off, cw) in enumerate(CHUNKS):
                    attention_chunk(0, off, cw)
                    qkv_pair(1, off, cw, "dve", PSF, PSF, "pf", "pf")
                    rs_chunk(1, off, cw)
                    norm_chunk(1, off, cw, rsb1p, nc.gpsimd)

                # hp1 attention; pair-0 1/s scaling rides in its window
                # (recip on DVE, broadcast via DRAM bounce, multiply).
                # same tag as the norm-broadcast tiles: rsbd0 reuses rsb0p's
                # buffer (dead after phase 1), rsbd1 reuses rsb1p's (dead
                # after the hp0 window).
                rsbd0 = RSB.tile([128, N], F32, tag="rsb", name="rsbd0")
                rsbd1 = RSB.tile([128, N], F32, tag="rsb", name="rsbd1")

                def scale_chunk(hp, off, cw, src, rsbd):
                    nc.vector.reciprocal_approx_fast(
                        out=rsden8[0:1, :, off:off + cw],
                        in_=src[0:1, :, off:off + cw])
                    t0 = 2 * hp
                    for t in range(2):
                        nc.sync.dma_start(
                            rsdd[t0 + t:t0 + t + 1, off:off + cw],
                            rsden8[0:1, t, off:off + cw])
                    bcast_row(rsdd[t0][off:off + cw],
                              rsbd[0:64, off:off + cw], 64)
                    bcast_row(rsdd[t0 + 1][off:off + cw],
                              rsbd[64:128, off:off + cw], 64)
                    nc.vector.tensor_mul(nsc[hp][:, off:off + cw],
                                         numer[hp][:, off:off + cw],
                                         rsbd[:, off:off + cw])

                for ci, (off, cw) in enumerate(CHUNKS):
                    attention_chunk(1, off, cw)
                    scale_chunk(0, off, cw, s8, rsbd0)

                # tail: merged output projection -- both head-pairs
                # accumulate into one PSUM bank per (m2, chunk); one
                # bias-add drain (bf16) + one store. Kept OUT of the
                # ACT-saturated attention window.
                def outproj_chunk(off, cw):
                    for m2 in range(2):
                        pf = PSF.tile([128, 512], F32, tag="pf", name="pf")
                        nc.tensor.matmul(
                            pf[:, 0:cw],
                            mc(woT_sb[0][:, m2 * 128:(m2 + 1) * 128]),
                            mc(nsc[0][:, off:off + cw]),
                            start=True, stop=False, skip_group_check=True)
                        nc.tensor.matmul(
                            pf[:, 0:cw],
                            mc(woT_sb[1][:, m2 * 128:(m2 + 1) * 128]),
                            mc(nsc[1][:, off:off + cw]),
                            start=False, stop=True, skip_group_check=True)
                        yt = YST.tile([128, 512], WD, tag="yt", name="yt")
                        nc.vector.tensor_scalar_add(
                            yt[:, 0:cw], pf[:, 0:cw], bias_sb[m2][:, :])
                        nc.sync.dma_start(y[m2][:, off:off + cw],
                                          yt[:, 0:cw])

                for (off, cw) in CHUNKS:
                    scale_chunk(1, off, cw, s8b, rsbd1)
                    outproj_chunk(off, cw)

    nc.compile()
    return nc


def _get_program(wd_name=WD_NAME):
    if wd_name not in _CACHE:
        _CACHE[wd_name] = _build(wd_name)
    return _CACHE[wd_name]


def _np_wd(wd_name):
    if wd_name == "bf16":
        import ml_dtypes
        return np.dtype(ml_dtypes.bfloat16)
    return np.dtype(np.float32)


def make_in_maps(x, w_qkv, w_out, b_out, wd_name=WD_NAME):
    x = np.asarray(x, np.float32)
    w_qkv = np.asarray(w_qkv, np.float32)
    w_out = np.asarray(w_out, np.float32)
    b_out = np.asarray(b_out, np.float32)
    wd = _np_wd(wd_name)

    ones8 = np.zeros((128, 9), np.float32)
    ones8[:, 8] = 1.0
    for cc in range(8):
        lo = 64 * (cc % 2)
        ones8[lo:lo + 64, cc] = 1.0

    in_maps = []
    for core in range(8):
        b, half = core // 2, core % 2
        hsel = slice(256 * half, 256 * (half + 1))
        q_rows = np.arange(0, 512)[hsel]
        k_rows = 512 + q_rows
        v_rows = 1024 + q_rows
        wqk_h = np.ascontiguousarray(
            w_qkv[np.r_[q_rows, k_rows], :].T).reshape(2, 128, 512)
        wvT_h = np.ascontiguousarray(w_qkv[v_rows, :].T).reshape(2, 128, 256)
        woT_h = np.ascontiguousarray(w_out[:, hsel].T).reshape(2, 128, 256)
        bias_h = (b_out if half == 0 else np.zeros_like(b_out))
        in_maps.append({
            "x2": x[b].reshape(C, N).reshape(2, 128, N).astype(wd),
            "wqk": wqk_h.astype(wd),
            "wvT": wvT_h.astype(wd),
            "woT": woT_h.astype(wd),
            "bias": bias_h.reshape(2, 128, 1).astype(np.float32),
            "ones8": ones8.astype(wd),
        })
    return in_maps


def gather_output(results):
    outs = [np.asarray(r["y"], dtype=np.float32).reshape(C, N)
            for r in results]
    return np.stack([
        (outs[2 * b] + outs[2 * b + 1]).reshape(C, H, W) for b in range(B)
    ]).astype(np.float32)


def run(in_maps, wd_name=WD_NAME, **kwargs):
    from concourse import bass_utils
    nc = _get_program(wd_name)
    return bass_utils.run_bass_kernel_spmd(nc, in_maps,
                                           core_ids=list(range(8)), **kwargs)


def kernel(x, w_qkv, w_out, b_out):
    in_maps = make_in_maps(x, w_qkv, w_out, b_out)
    res = run(in_maps)
    return gather_output(res.results)
